# revision 20
# baseline (speedup 1.0000x reference)
"""Causal self-attention (GQA, rope, qk-rmsnorm) on 8 TRN2 NeuronCores.

Sharding: core = (b, g), b = core // 4 (batch), g = core % 4.
Each core owns 8 interleaved 64-row query slots of its batch (balanced
causal assignment), computes Q for those 512 rows (all 16 heads), K/V
for kv-head g only (sharded 4-way), all-gathers K/V within its 4-core
batch group (overlapped with the Q projection), runs attention for all
heads over its own queries, and projects its 512-row output slice
locally (no output collective).

All PE-facing tensors stay transposed ([feature, token]); rope's
cross-partition half-swap runs on the PE via two constant combine
matrices, rms-norm partition sums via a ones matmul, and the causal
mask is a post-exp 0/1 multiply on the vector engine. The host
pre-transposes x and converts weights to bf16.

Engine queues: sync = pure loads, scalar = activation compute (+ the
v DMA-transposes), gpsimd = dependent stores, the collective, gather-
ins, and the den pair-adds.
"""

import sys

if "/opt/trn_rl_repo" not in sys.path:
    sys.path.insert(0, "/opt/trn_rl_repo")

import numpy as np
import ml_dtypes

BF16 = ml_dtypes.bfloat16

B, T, C = 2, 2048, 2048
NH, NKV = 16, 4
HD = 128
P = 128
NCT = C // P           # 16 contraction tiles
QR = 512               # own query rows per core
NKT = T // P           # 16 key tiles
A_SHAPE = [16, 14, 12, 10, 8, 6, 4, 2]  # uniform key-tile count per slot
EPS = float(np.finfo(np.float32).eps)
USE_DSQRT = False      # Dsqrt has no ACT table on this stack; use Sqrt+recip

_CACHE = {}


def _slots64(g):
    """Own 64-row query chunks, descending. Balanced: sum of causal
    key-needs is equal across g."""
    return [31 - g, 24 + g, 23 - g, 16 + g, 15 - g, 8 + g, 7 - g, g]


def _qrows(g):
    return np.concatenate([np.arange(c * 64, (c + 1) * 64) for c in _slots64(g)])


def _mask01(g):
    """Additive causal mask, accumulated into the score psum on the PE:
    for slot i the program masks key tiles A_SHAPE[i]-2 and A_SHAPE[i]-1
    (j = 0, 1); entry is 0 where key <= query else -1e9."""
    m = np.zeros((8, 2, P, 64), np.float32)
    sl = _slots64(g)
    for i in range(8):
        q = sl[i] * 64 + np.arange(64)[None, :]
        for j in range(2):
            kt = A_SHAPE[i] - 2 + j
            k = kt * P + np.arange(P)[:, None]
            m[i, j] = np.where(k <= q, 0.0, -1.0e9)
    return np.ascontiguousarray(m.astype(BF16))


def _rope_mats():
    """ro = A @ m1 + B @ m2 with m1 = q*[cos;sin], m2 = q*[sin;cos]:
    ro[0:64] = m1[0:64] + m1[64:128]; ro[64:128] = m2[64:128] - m2[0:64].
    Returned transposed ([contraction, out_partition]) for use as lhsT."""
    Am = np.zeros((P, P), np.float32)
    Bm = np.zeros((P, P), np.float32)
    for o in range(64):
        Am[o, o] = 1.0
        Am[o + 64, o] = 1.0
    for o in range(64, 128):
        Bm[o, o] = 1.0
        Bm[o - 64, o] = -1.0
    return np.ascontiguousarray(Am.astype(BF16)), np.ascontiguousarray(Bm.astype(BF16))


def _build():
    import concourse.bacc as bacc
    import concourse.mybir as mybir
    import concourse.tile as tile
    from concourse.masks import make_identity

    f32 = mybir.dt.float32
    bf16 = mybir.dt.bfloat16
    AF = mybir.ActivationFunctionType
    OP = mybir.AluOpType

    nc = bacc.Bacc("TRN2", target_bir_lowering=False, debug=False, num_devices=8)

    xoT = nc.dram_tensor("xoT", [P, NCT, QR], bf16, kind="ExternalInput").ap()
    xfT = nc.dram_tensor("xfT", [4, P, NCT, QR], bf16, kind="ExternalInput").ap()
    cso = nc.dram_tensor("cso", [P, QR], bf16, kind="ExternalInput").ap()
    sno = nc.dram_tensor("sno", [P, QR], bf16, kind="ExternalInput").ap()
    csf = nc.dram_tensor("csf", [P, T], bf16, kind="ExternalInput").ap()
    snf = nc.dram_tensor("snf", [P, T], bf16, kind="ExternalInput").ap()
    wq = nc.dram_tensor("wq", [NH, P, NCT, HD], bf16, kind="ExternalInput").ap()
    wks = nc.dram_tensor("wks", [P, NCT, HD], bf16, kind="ExternalInput").ap()
    wvs = nc.dram_tensor("wvs", [P, NCT, HD], bf16, kind="ExternalInput").ap()
    wo = nc.dram_tensor("wo", [4, P, NCT, QR], bf16, kind="ExternalInput").ap()
    msk = nc.dram_tensor("msk", [P, 8, 2, 64], bf16, kind="ExternalInput").ap()
    rpA = nc.dram_tensor("rpA", [P, P], bf16, kind="ExternalInput").ap()
    rpB = nc.dram_tensor("rpB", [P, P], bf16, kind="ExternalInput").ap()
    yo = nc.dram_tensor("yo", [QR, C], f32, kind="ExternalOutput").ap()

    with tile.TileContext(nc) as tc:
        with (
            tc.tile_pool(name="singles", bufs=1) as singles,
            tc.tile_pool(name="big", bufs=1) as bigpool,
            tc.tile_pool(name="xc", bufs=3) as xcpool,
            tc.tile_pool(name="wqh", bufs=6) as wqpool,
            tc.tile_pool(name="rope", bufs=3) as ropep,
            tc.tile_pool(name="stage", bufs=2) as stagep,
            tc.tile_pool(name="pt", bufs=6) as ptpool,
            tc.tile_pool(name="padd", bufs=3) as paddp,
            tc.tile_pool(name="small", bufs=2) as smallp,
            tc.tile_pool(name="outs", bufs=2) as outpool,
            tc.tile_pool(name="psS", bufs=4, space="PSUM") as psS,
            tc.tile_pool(name="psY", bufs=2, space="PSUM") as psY,
            tc.tile_pool(name="psD", bufs=2, space="PSUM") as psD,
            tc.tile_pool(name="dram", bufs=1, space="DRAM") as drampool,
        ):
            ones128 = singles.tile([P, P], bf16)
            nc.vector.memset(ones128, 1.0)
            ident = singles.tile([P, P], bf16)
            make_identity(nc, ident)
            eps_q = singles.tile([P, 1], f32)
            nc.vector.memset(eps_q, HD * EPS / 4.0 if USE_DSQRT else HD * EPS)
            eps_k = singles.tile([P, 1], f32)
            nc.vector.memset(eps_k, EPS / 4.0 if USE_DSQRT else EPS)
            sc_q = 0.25 if USE_DSQRT else 1.0
            sc_k = (1.0 / (4.0 * HD)) if USE_DSQRT else 1.0 / HD
            # load order matters: sync is FIFO — K/V weights and the first
            # x chunk first so the PE starts ASAP; cos/sin split per chunk
            wks_sb = singles.tile([P, NCT, HD], bf16)
            nc.sync.dma_start(out=wks_sb, in_=wks)
            wvs_sb = singles.tile([P, NCT, HD], bf16)
            nc.sync.dma_start(out=wvs_sb, in_=wvs)
            rpA_sb = singles.tile([P, P], bf16)
            rpB_sb = singles.tile([P, P], bf16)
            csf_sb = singles.tile([P, T], bf16)
            snf_sb = singles.tile([P, T], bf16)
            cso_sb = singles.tile([P, QR], bf16)
            sno_sb = singles.tile([P, QR], bf16)
            mk = singles.tile([P, 8, 2, 64], bf16)

            kv_in = drampool.tile([P, 2 * T], bf16, tag="kvin")
            kv_out = drampool.tile([NKV * P, 2 * T], bf16, tag="kvout")

            def rope_rms(ps, cs_ap, sn_ap, out_ap, scale, bias_ap, nm):
                """ps: [128, 512] psum f32 = projected [head_dim, tok].
                Applies rope (PE combine) + rms-norm, writes bf16 out_ap."""
                m1 = ropep.tile([P, QR], bf16, tag="m1", name=f"m1{nm}")
                m2 = ropep.tile([P, QR], bf16, tag="m2", name=f"m2{nm}")
                nc.vector.tensor_tensor(m1, ps, cs_ap, op=OP.mult)
                nc.vector.tensor_tensor(m2, ps, sn_ap, op=OP.mult)
                ro = psY.tile([P, QR], f32, tag="Y", name=f"ro{nm}")
                nc.tensor.matmul(ro, rpA_sb, m1, start=True, stop=False)
                nc.tensor.matmul(ro, rpB_sb, m2, start=False, stop=True)
                sq = ropep.tile([P, QR], bf16, tag="sq", name=f"sq{nm}")
                nc.scalar.activation(sq, ro, AF.Square)
                ss = psD.tile([P, QR], f32, tag="D", name=f"ss{nm}")
                nc.tensor.matmul(ss, ones128, sq, start=True, stop=True)
                rinv = ropep.tile([P, QR], f32, tag="rinv", name=f"rinv{nm}")
                if USE_DSQRT:
                    nc.scalar.activation(rinv, ss, AF.Dsqrt, bias=bias_ap, scale=scale)
                else:
                    rms = ropep.tile([P, QR], f32, tag="rms", name=f"rms{nm}")
                    nc.scalar.activation(rms, ss, AF.Sqrt, bias=bias_ap, scale=scale)
                    nc.vector.reciprocal_approx_fast(rinv, rms)
                nc.vector.tensor_tensor(out_ap, ro, rinv, op=OP.mult)

            # ---------------- phase 1: K/V projection (own kv head) -----
            for j in range(4):
                xc = xcpool.tile([P, NCT, QR], bf16, tag="xc", name=f"xc{j}")
                for hf in range(2):
                    nc.sync.dma_start(
                        out=xc[:, 8 * hf:8 * hf + 8, :],
                        in_=xfT[j][:, 8 * hf:8 * hf + 8, :],
                    )
                if j == 0:
                    nc.sync.dma_start(out=rpA_sb, in_=rpA)
                    nc.sync.dma_start(out=rpB_sb, in_=rpB)
                nc.sync.dma_start(
                    out=csf_sb[:, j * QR:(j + 1) * QR],
                    in_=csf[:, j * QR:(j + 1) * QR],
                )
                nc.sync.dma_start(
                    out=snf_sb[:, j * QR:(j + 1) * QR],
                    in_=snf[:, j * QR:(j + 1) * QR],
                )
                kp = psS.tile([P, QR], f32, tag="S", name=f"kp{j}")
                for ct in range(NCT):
                    nc.tensor.matmul(
                        kp, wks_sb[:, ct, :], xc[:, ct, :],
                        start=(ct == 0), stop=(ct == NCT - 1),
                    )
                kst = stagep.tile([P, QR], bf16, tag="kst", name=f"kst{j}")
                rope_rms(
                    kp, csf_sb[:, j * QR:(j + 1) * QR],
                    snf_sb[:, j * QR:(j + 1) * QR], kst, sc_k, eps_k, f"k{j}",
                )
                nc.gpsimd.dma_start(out=kv_in[:, j * QR:(j + 1) * QR], in_=kst)

                vp = psS.tile([P, QR], f32, tag="S", name=f"vp{j}")
                for ct in range(NCT):
                    nc.tensor.matmul(
                        vp, wvs_sb[:, ct, :], xc[:, ct, :],
                        start=(ct == 0), stop=(ct == NCT - 1),
                    )
                vtmp = stagep.tile([P, QR], bf16, tag="vtmp", name=f"vtmp{j}")
                nc.vector.tensor_copy(vtmp, vp)
                vstage = stagep.tile([P, 4, P], bf16, tag="vst", name=f"vst{j}")
                for tk in range(4):
                    nc.scalar.dma_start_transpose(
                        vstage[:, tk, :], vtmp[:, tk * P:(tk + 1) * P]
                    )
                nc.gpsimd.dma_start(
                    out=kv_in[:, T + j * QR:T + (j + 1) * QR], in_=vstage
                )

            nc.gpsimd.collective_compute(
                "AllGather",
                OP.bypass,
                replica_groups=[[0, 1, 2, 3], [4, 5, 6, 7]],
                ins=[kv_in.opt()],
                outs=[kv_out.opt()],
            )

            # K/V gathered from the group (gpsimd: right after the trigger)
            kT = bigpool.tile([P, NKV, T], bf16, tag="kT")
            vA = bigpool.tile([P, NKV, NKT, HD], bf16, tag="vA")
            for kv in range(NKV):
                nc.gpsimd.dma_start(
                    out=kT[:, kv, :], in_=kv_out[kv * P:(kv + 1) * P, 0:T]
                )
                nc.gpsimd.dma_start(
                    out=vA[:, kv, :, :],
                    in_=kv_out[kv * P:(kv + 1) * P, T:2 * T].rearrange(
                        "p (t d) -> p t d", d=HD
                    ),
                )

            # ---------------- phase 2: Q projection (all heads) ---------
            nc.sync.dma_start(out=cso_sb, in_=cso)
            nc.sync.dma_start(out=sno_sb, in_=sno)
            nc.sync.dma_start(out=mk, in_=msk)
            # shares the xc pool (xc chunks are dead by now)
            xo_sb = xcpool.tile([P, NCT, QR], bf16, tag="xc", name="xo")
            for hf in range(4):
                nc.sync.dma_start(
                    out=xo_sb[:, 4 * hf:4 * hf + 4, :],
                    in_=xoT[:, 4 * hf:4 * hf + 4, :],
                )
            qT = bigpool.tile([P, NH, QR], bf16, tag="qT")
            for h in range(NH):
                wqh = wqpool.tile([P, NCT, HD], bf16, tag="wqh", name=f"wqh{h}")
                nc.sync.dma_start(out=wqh, in_=wq[h])
                qp = psS.tile([P, QR], f32, tag="S", name=f"qp{h}")
                for ct in range(NCT):
                    nc.tensor.matmul(
                        qp, wqh[:, ct, :], xo_sb[:, ct, :],
                        start=(ct == 0), stop=(ct == NCT - 1),
                    )
                rope_rms(qp, cso_sb, sno_sb, qT[:, h, :], sc_q, eps_q, f"q{h}")

            # ---------------- phase 3: attention ------------------------
            yT = bigpool.tile([P, NH, QR], bf16, tag="yT")

            def load_wo_slab(s):
                # shares the xc pool; slab s=1 evicts xo_sb (read-dep safe)
                w3 = xcpool.tile([P, NCT, QR], bf16, tag="xc", name=f"wo{s}")
                for gr in range(4):
                    nc.sync.dma_start(
                        out=w3[:, 4 * gr:4 * gr + 4, :],
                        in_=wo[s][:, 4 * gr:4 * gr + 4, :],
                    )
                return w3

            w3s = {0: load_wo_slab(0), 1: load_wo_slab(1)}

            tail = []

            def emit_tail():
                yt, den, h = tail.pop(0)
                rinv = smallp.tile([P, QR], f32, tag="rq", name=f"rq{h}")
                nc.vector.reciprocal_approx_fast(rinv, den)
                nc.vector.tensor_tensor(yT[:, h, :], yt, rinv, op=OP.mult)

            def emit_ent(ent, yt, den, kv):
                """single: per-kt den + PV; pair: pair-summed den + 2 PV."""
                kind, m, n, pa, pb = ent
                if kind == "s":
                    kt = m
                    last = False
                    nc.tensor.matmul(
                        den[:, 0:n], ones128, pa,
                        start=(kt == 0), stop=False, skip_group_check=True,
                    )
                    nc.tensor.matmul(
                        yt[:, 0:n], vA[:, kv, kt, :], pa,
                        start=(kt == 0), stop=False, skip_group_check=True,
                    )
                else:
                    last = m == 7
                    pad = paddp.tile([P, QR], bf16, tag="pa", name=f"pad{m}")
                    nc.vector.tensor_tensor(pad[:, 0:n], pa, pb, op=OP.add)
                    nc.tensor.matmul(
                        den[:, 0:n], ones128, pad[:, 0:n],
                        start=False, stop=last, skip_group_check=True,
                    )
                    nc.tensor.matmul(
                        yt[:, 0:n], vA[:, kv, 2 * m, :], pa,
                        start=False, stop=False, skip_group_check=True,
                    )
                    nc.tensor.matmul(
                        yt[:, 0:n], vA[:, kv, 2 * m + 1, :], pb,
                        start=False, stop=last, skip_group_check=True,
                    )

            for h in range(NH):
                kv = h // (NH // NKV)
                yt = psY.tile([P, QR], f32, tag="Y", name=f"yt{h}")
                den = psD.tile([P, QR], f32, tag="D", name=f"den{h}")
                dq = []
                # kt 0..7: one kt per psum tile, causal mask accumulated
                # on the PE, pipeline depth 4
                for kt in range(8):
                    n = 64 * (8 - kt // 2)
                    i = (15 - kt) // 2
                    S = psS.tile([P, QR], f32, tag="S", name=f"S{h}_{kt}")
                    nc.tensor.matmul(
                        S[:, 0:n],
                        kT[:, kv, kt * P:(kt + 1) * P],
                        qT[:, h, 0:n],
                        start=True, stop=False, skip_group_check=True,
                    )
                    nc.tensor.matmul(
                        S[:, n - 64:n], ident, mk[:, i, kt % 2, :],
                        start=False, stop=True, skip_group_check=True,
                    )
                    if kt == 0 and tail:
                        emit_tail()
                    pt = ptpool.tile([P, QR], bf16, tag="pt", name=f"pt{h}_{kt}")
                    nc.scalar.activation(pt[:, 0:n], S[:, 0:n], AF.Exp)
                    dq.append(("s", kt, n, pt[:, 0:n], None))
                    if len(dq) > 3:
                        emit_ent(dq.pop(0), yt, den, kv)
                # kt 8..15: both kt of a pair packed into one psum bank
                for m in range(4, 8):
                    n = 64 * (8 - m)
                    i = 7 - m
                    S = psS.tile([P, QR], f32, tag="S", name=f"S{h}_{m}p")
                    for t in range(2):
                        nc.tensor.matmul(
                            S[:, t * n:(t + 1) * n],
                            kT[:, kv, (2 * m + t) * P:(2 * m + t + 1) * P],
                            qT[:, h, 0:n],
                            start=True, stop=False, skip_group_check=True,
                        )
                        nc.tensor.matmul(
                            S[:, (t + 1) * n - 64:(t + 1) * n], ident,
                            mk[:, i, t, :],
                            start=False, stop=True, skip_group_check=True,
                        )
                    pt = ptpool.tile([P, QR], bf16, tag="pt", name=f"pt{h}_{m}p")
                    nc.scalar.activation(pt[:, 0:2 * n], S[:, 0:2 * n], AF.Exp)
                    dq.append(("p", m, n, pt[:, 0:n], pt[:, n:2 * n]))
                    if len(dq) > 3:
                        emit_ent(dq.pop(0), yt, den, kv)
                while dq:
                    emit_ent(dq.pop(0), yt, den, kv)
                tail.append((yt, den, h))
            emit_tail()

            # ---------------- phase 4: output projection ----------------
            for s in range(4):
                w3 = w3s.pop(s)
                if s + 2 < 4:
                    w3s[s + 2] = load_wo_slab(s + 2)
                for rt in range(4):
                    ps = psS.tile([P, QR], f32, tag="S", name=f"o{s}_{rt}")
                    for h in range(NH):
                        nc.tensor.matmul(
                            ps, yT[:, h, rt * P:(rt + 1) * P], w3[:, h, :],
                            start=(h == 0), stop=(h == NH - 1),
                        )
                    ot = outpool.tile([P, QR], f32, tag="ot", name=f"ot{s}_{rt}")
                    nc.vector.tensor_copy(ot, ps)
                    nc.gpsimd.dma_start(
                        out=yo[rt * P:(rt + 1) * P, s * QR:(s + 1) * QR], in_=ot
                    )

    nc.compile()
    return nc


def _get_nc():
    if "nc" not in _CACHE:
        _CACHE["nc"] = _build()
    return _CACHE["nc"]


def _prep_in_maps(x, cos, sin, wq, wk, wv, wo):
    x = np.asarray(x, np.float32)
    cosr = np.asarray(cos, np.float32).reshape(T, HD // 2)
    sinr = np.asarray(sin, np.float32).reshape(T, HD // 2)
    # weight layouts match the SBUF tiles exactly -> contiguous DMAs
    wqb = np.ascontiguousarray(
        np.asarray(wq, np.float32).reshape(NCT, P, NH, HD)
        .transpose(2, 1, 0, 3).astype(BF16))
    wob = np.ascontiguousarray(
        np.asarray(wo, np.float32).reshape(NCT, P, 4, QR)
        .transpose(2, 1, 0, 3).astype(BF16))
    wkf = np.asarray(wk, np.float32)
    wvf = np.asarray(wv, np.float32)

    csf = np.ascontiguousarray(np.concatenate([cosr.T, sinr.T], axis=0).astype(BF16))
    snf = np.ascontiguousarray(np.concatenate([sinr.T, cosr.T], axis=0).astype(BF16))
    rpA_np, rpB_np = _rope_mats()

    maps = []
    for core in range(8):
        b, g = core // 4, core % 4
        qr = _qrows(g)
        xT_b = x[b].T
        maps.append({
            "xoT": np.ascontiguousarray(
                xT_b[:, qr].reshape(NCT, P, QR).transpose(1, 0, 2).astype(BF16)),
            "xfT": np.ascontiguousarray(
                xT_b.reshape(NCT, P, 4, QR).transpose(2, 1, 0, 3).astype(BF16)),
            "cso": np.ascontiguousarray(csf[:, qr]),
            "sno": np.ascontiguousarray(snf[:, qr]),
            "csf": csf,
            "snf": snf,
            "wq": wqb,
            "wks": np.ascontiguousarray(
                wkf[:, g * HD:(g + 1) * HD].reshape(NCT, P, HD)
                .transpose(1, 0, 2).astype(BF16)),
            "wvs": np.ascontiguousarray(
                wvf[:, g * HD:(g + 1) * HD].reshape(NCT, P, HD)
                .transpose(1, 0, 2).astype(BF16)),
            "wo": wob,
            "msk": np.ascontiguousarray(_mask01(g).transpose(2, 0, 1, 3)),
            "rpA": rpA_np,
            "rpB": rpB_np,
        })
    return maps


def kernel(x, cos, sin, wq, wk, wv, wo):
    from concourse.bass_utils import run_bass_kernel_spmd

    nc = _get_nc()
    maps = _prep_in_maps(x, cos, sin, wq, wk, wv, wo)
    _CACHE["in_maps"] = maps
    res = run_bass_kernel_spmd(nc, maps, list(range(8)))
    y = np.empty((B, T, C), np.float32)
    for core in range(8):
        b, g = core // 4, core % 4
        y[b][_qrows(g)] = res.results[core]["yo"]
    return y


# revision 21
# speedup vs baseline: 1.0125x; 1.0125x over previous
"""Causal self-attention (GQA, rope, qk-rmsnorm) on 8 TRN2 NeuronCores.

Sharding: core = (b, g), b = core // 4 (batch), g = core % 4.
Each core owns 8 interleaved 64-row query slots of its batch (balanced
causal assignment), computes Q for those 512 rows (all 16 heads), K/V
for kv-head g only (sharded 4-way), all-gathers K/V within its 4-core
batch group (overlapped with the Q projection), runs attention for all
heads over its own queries, and projects its 512-row output slice
locally (no output collective).

All PE-facing tensors stay transposed ([feature, token]); rope's
cross-partition half-swap runs on the PE via two constant combine
matrices, rms-norm partition sums via a ones matmul, and the causal
mask is a post-exp 0/1 multiply on the vector engine. The host
pre-transposes x and converts weights to bf16.

Engine queues: sync = pure loads, scalar = activation compute (+ the
v DMA-transposes), gpsimd = dependent stores, the collective, gather-
ins, and the den pair-adds.
"""

import sys

if "/opt/trn_rl_repo" not in sys.path:
    sys.path.insert(0, "/opt/trn_rl_repo")

import numpy as np
import ml_dtypes

BF16 = ml_dtypes.bfloat16

B, T, C = 2, 2048, 2048
NH, NKV = 16, 4
HD = 128
P = 128
NCT = C // P           # 16 contraction tiles
QR = 512               # own query rows per core
NKT = T // P           # 16 key tiles
A_SHAPE = [16, 14, 12, 10, 8, 6, 4, 2]  # uniform key-tile count per slot
EPS = float(np.finfo(np.float32).eps)
USE_DSQRT = False      # Dsqrt has no ACT table on this stack; use Sqrt+recip

_CACHE = {}


def _slots64(g):
    """Own 64-row query chunks, descending. Balanced: sum of causal
    key-needs is equal across g."""
    return [31 - g, 24 + g, 23 - g, 16 + g, 15 - g, 8 + g, 7 - g, g]


def _qrows(g):
    return np.concatenate([np.arange(c * 64, (c + 1) * 64) for c in _slots64(g)])


def _mask01(g):
    """Additive causal mask, accumulated into the score psum on the PE:
    for slot i the program masks key tiles A_SHAPE[i]-2 and A_SHAPE[i]-1
    (j = 0, 1); entry is 0 where key <= query else -1e9."""
    m = np.zeros((8, 2, P, 64), np.float32)
    sl = _slots64(g)
    for i in range(8):
        q = sl[i] * 64 + np.arange(64)[None, :]
        for j in range(2):
            kt = A_SHAPE[i] - 2 + j
            k = kt * P + np.arange(P)[:, None]
            m[i, j] = np.where(k <= q, 0.0, -1.0e9)
    return np.ascontiguousarray(m.astype(BF16))


def _rope_mats():
    """ro = A @ m1 + B @ m2 with m1 = q*[cos;sin], m2 = q*[sin;cos]:
    ro[0:64] = m1[0:64] + m1[64:128]; ro[64:128] = m2[64:128] - m2[0:64].
    Returned transposed ([contraction, out_partition]) for use as lhsT."""
    Am = np.zeros((P, P), np.float32)
    Bm = np.zeros((P, P), np.float32)
    for o in range(64):
        Am[o, o] = 1.0
        Am[o + 64, o] = 1.0
    for o in range(64, 128):
        Bm[o, o] = 1.0
        Bm[o - 64, o] = -1.0
    return np.ascontiguousarray(Am.astype(BF16)), np.ascontiguousarray(Bm.astype(BF16))


def _build():
    import concourse.bacc as bacc
    import concourse.mybir as mybir
    import concourse.tile as tile
    from concourse.masks import make_identity

    f32 = mybir.dt.float32
    bf16 = mybir.dt.bfloat16
    AF = mybir.ActivationFunctionType
    OP = mybir.AluOpType

    nc = bacc.Bacc("TRN2", target_bir_lowering=False, debug=False, num_devices=8)

    xoT = nc.dram_tensor("xoT", [P, NCT, QR], bf16, kind="ExternalInput").ap()
    xfT = nc.dram_tensor("xfT", [4, P, NCT, QR], bf16, kind="ExternalInput").ap()
    cso = nc.dram_tensor("cso", [P, QR], bf16, kind="ExternalInput").ap()
    sno = nc.dram_tensor("sno", [P, QR], bf16, kind="ExternalInput").ap()
    csf = nc.dram_tensor("csf", [P, T], bf16, kind="ExternalInput").ap()
    snf = nc.dram_tensor("snf", [P, T], bf16, kind="ExternalInput").ap()
    wq = nc.dram_tensor("wq", [NH, P, NCT, HD], bf16, kind="ExternalInput").ap()
    wks = nc.dram_tensor("wks", [P, NCT, HD], bf16, kind="ExternalInput").ap()
    wvs = nc.dram_tensor("wvs", [P, NCT, HD], bf16, kind="ExternalInput").ap()
    wo = nc.dram_tensor("wo", [4, P, NCT, QR], bf16, kind="ExternalInput").ap()
    msk = nc.dram_tensor("msk", [P, 8, 2, 64], bf16, kind="ExternalInput").ap()
    rpA = nc.dram_tensor("rpA", [P, P], bf16, kind="ExternalInput").ap()
    rpB = nc.dram_tensor("rpB", [P, P], bf16, kind="ExternalInput").ap()
    yo = nc.dram_tensor("yo", [QR, C], f32, kind="ExternalOutput").ap()

    with tile.TileContext(nc) as tc:
        with (
            tc.tile_pool(name="singles", bufs=1) as singles,
            tc.tile_pool(name="big", bufs=1) as bigpool,
            tc.tile_pool(name="xc", bufs=2) as xcpool,
            tc.tile_pool(name="wqh", bufs=8) as wqpool,
            tc.tile_pool(name="rope", bufs=3) as ropep,
            tc.tile_pool(name="stage", bufs=2) as stagep,
            tc.tile_pool(name="pt", bufs=6) as ptpool,
            tc.tile_pool(name="padd", bufs=3) as paddp,
            tc.tile_pool(name="small", bufs=2) as smallp,
            tc.tile_pool(name="outs", bufs=2) as outpool,
            tc.tile_pool(name="psS", bufs=4, space="PSUM") as psS,
            tc.tile_pool(name="psY", bufs=2, space="PSUM") as psY,
            tc.tile_pool(name="psD", bufs=2, space="PSUM") as psD,
            tc.tile_pool(name="dram", bufs=1, space="DRAM") as drampool,
        ):
            ones128 = singles.tile([P, P], bf16)
            nc.vector.memset(ones128, 1.0)
            ident = singles.tile([P, P], bf16)
            make_identity(nc, ident)
            eps_q = singles.tile([P, 1], f32)
            nc.vector.memset(eps_q, HD * EPS / 4.0 if USE_DSQRT else HD * EPS)
            eps_k = singles.tile([P, 1], f32)
            nc.vector.memset(eps_k, EPS / 4.0 if USE_DSQRT else EPS)
            sc_q = 0.25 if USE_DSQRT else 1.0
            sc_k = (1.0 / (4.0 * HD)) if USE_DSQRT else 1.0 / HD
            # load order matters: sync is FIFO — K/V weights and the first
            # x chunk first so the PE starts ASAP; cos/sin split per chunk
            wks_sb = singles.tile([P, NCT, HD], bf16)
            nc.sync.dma_start(out=wks_sb, in_=wks)
            wvs_sb = singles.tile([P, NCT, HD], bf16)
            nc.sync.dma_start(out=wvs_sb, in_=wvs)
            rpA_sb = singles.tile([P, P], bf16)
            rpB_sb = singles.tile([P, P], bf16)
            csf_sb = singles.tile([P, T], bf16)
            snf_sb = singles.tile([P, T], bf16)
            cso_sb = singles.tile([P, QR], bf16)
            sno_sb = singles.tile([P, QR], bf16)
            mk = singles.tile([P, 8, 2, 64], bf16)

            kv_in = drampool.tile([P, 2 * T], bf16, tag="kvin")
            kv_out = drampool.tile([NKV * P, 2 * T], bf16, tag="kvout")

            def rope_rms(ps, cs_ap, sn_ap, out_ap, scale, bias_ap, nm):
                """ps: [128, 512] psum f32 = projected [head_dim, tok].
                Applies rope (PE combine) + rms-norm, writes bf16 out_ap."""
                m1 = ropep.tile([P, QR], bf16, tag="m1", name=f"m1{nm}")
                m2 = ropep.tile([P, QR], bf16, tag="m2", name=f"m2{nm}")
                nc.vector.tensor_tensor(m1, ps, cs_ap, op=OP.mult)
                nc.vector.tensor_tensor(m2, ps, sn_ap, op=OP.mult)
                ro = psY.tile([P, QR], f32, tag="Y", name=f"ro{nm}")
                nc.tensor.matmul(ro, rpA_sb, m1, start=True, stop=False)
                nc.tensor.matmul(ro, rpB_sb, m2, start=False, stop=True)
                sq = ropep.tile([P, QR], bf16, tag="sq", name=f"sq{nm}")
                nc.scalar.activation(sq, ro, AF.Square)
                ss = psD.tile([P, QR], f32, tag="D", name=f"ss{nm}")
                nc.tensor.matmul(ss, ones128, sq, start=True, stop=True)
                rinv = ropep.tile([P, QR], f32, tag="rinv", name=f"rinv{nm}")
                if USE_DSQRT:
                    nc.scalar.activation(rinv, ss, AF.Dsqrt, bias=bias_ap, scale=scale)
                else:
                    rms = ropep.tile([P, QR], f32, tag="rms", name=f"rms{nm}")
                    nc.scalar.activation(rms, ss, AF.Sqrt, bias=bias_ap, scale=scale)
                    nc.vector.reciprocal_approx_fast(rinv, rms)
                nc.vector.tensor_tensor(out_ap, ro, rinv, op=OP.mult)

            # ---------------- phase 1: K/V projection (own kv head) -----
            for j in range(4):
                xc = xcpool.tile([P, NCT, QR], bf16, tag="xc", name=f"xc{j}")
                for hf in range(2):
                    nc.sync.dma_start(
                        out=xc[:, 8 * hf:8 * hf + 8, :],
                        in_=xfT[j][:, 8 * hf:8 * hf + 8, :],
                    )
                if j == 0:
                    nc.sync.dma_start(out=rpA_sb, in_=rpA)
                    nc.sync.dma_start(out=rpB_sb, in_=rpB)
                nc.sync.dma_start(
                    out=csf_sb[:, j * QR:(j + 1) * QR],
                    in_=csf[:, j * QR:(j + 1) * QR],
                )
                nc.sync.dma_start(
                    out=snf_sb[:, j * QR:(j + 1) * QR],
                    in_=snf[:, j * QR:(j + 1) * QR],
                )
                kp = psS.tile([P, QR], f32, tag="S", name=f"kp{j}")
                for ct in range(NCT):
                    nc.tensor.matmul(
                        kp, wks_sb[:, ct, :], xc[:, ct, :],
                        start=(ct == 0), stop=(ct == NCT - 1),
                    )
                kst = stagep.tile([P, QR], bf16, tag="kst", name=f"kst{j}")
                rope_rms(
                    kp, csf_sb[:, j * QR:(j + 1) * QR],
                    snf_sb[:, j * QR:(j + 1) * QR], kst, sc_k, eps_k, f"k{j}",
                )
                nc.gpsimd.dma_start(out=kv_in[:, j * QR:(j + 1) * QR], in_=kst)

                vp = psS.tile([P, QR], f32, tag="S", name=f"vp{j}")
                for ct in range(NCT):
                    nc.tensor.matmul(
                        vp, wvs_sb[:, ct, :], xc[:, ct, :],
                        start=(ct == 0), stop=(ct == NCT - 1),
                    )
                vtmp = stagep.tile([P, QR], bf16, tag="vtmp", name=f"vtmp{j}")
                nc.vector.tensor_copy(vtmp, vp)
                vstage = stagep.tile([P, 4, P], bf16, tag="vst", name=f"vst{j}")
                for tk in range(4):
                    nc.scalar.dma_start_transpose(
                        vstage[:, tk, :], vtmp[:, tk * P:(tk + 1) * P]
                    )
                nc.gpsimd.dma_start(
                    out=kv_in[:, T + j * QR:T + (j + 1) * QR], in_=vstage
                )

            nc.gpsimd.collective_compute(
                "AllGather",
                OP.bypass,
                replica_groups=[[0, 1, 2, 3], [4, 5, 6, 7]],
                ins=[kv_in.opt()],
                outs=[kv_out.opt()],
            )

            # K/V gathered from the group (gpsimd: right after the trigger)
            kT = bigpool.tile([P, NKV, T], bf16, tag="kT")
            vA = bigpool.tile([P, NKV, NKT, HD], bf16, tag="vA")
            for kv in range(NKV):
                nc.gpsimd.dma_start(
                    out=kT[:, kv, :], in_=kv_out[kv * P:(kv + 1) * P, 0:T]
                )
                nc.gpsimd.dma_start(
                    out=vA[:, kv, :, :],
                    in_=kv_out[kv * P:(kv + 1) * P, T:2 * T].rearrange(
                        "p (t d) -> p t d", d=HD
                    ),
                )

            # ---------------- phase 2: Q projection (all heads) ---------
            nc.sync.dma_start(out=cso_sb, in_=cso)
            nc.sync.dma_start(out=sno_sb, in_=sno)
            nc.sync.dma_start(out=mk, in_=msk)
            # shares the xc pool (xc chunks are dead by now)
            xo_sb = xcpool.tile([P, NCT, QR], bf16, tag="xc", name="xo")
            for hf in range(4):
                nc.sync.dma_start(
                    out=xo_sb[:, 4 * hf:4 * hf + 4, :],
                    in_=xoT[:, 4 * hf:4 * hf + 4, :],
                )
            qT = bigpool.tile([P, NH, QR], bf16, tag="qT")
            for h in range(NH):
                wqh = wqpool.tile([P, NCT, HD], bf16, tag="wqh", name=f"wqh{h}")
                nc.sync.dma_start(out=wqh, in_=wq[h])
                qp = psS.tile([P, QR], f32, tag="S", name=f"qp{h}")
                for ct in range(NCT):
                    nc.tensor.matmul(
                        qp, wqh[:, ct, :], xo_sb[:, ct, :],
                        start=(ct == 0), stop=(ct == NCT - 1),
                    )
                rope_rms(qp, cso_sb, sno_sb, qT[:, h, :], sc_q, eps_q, f"q{h}")

            # ---------------- phase 3: attention ------------------------
            yT = bigpool.tile([P, NH, QR], bf16, tag="yT")

            def load_wo_slab(s):
                # shares the xc pool; slab s=1 evicts xo_sb (read-dep safe)
                w3 = xcpool.tile([P, NCT, QR], bf16, tag="xc", name=f"wo{s}")
                for gr in range(4):
                    nc.sync.dma_start(
                        out=w3[:, 4 * gr:4 * gr + 4, :],
                        in_=wo[s][:, 4 * gr:4 * gr + 4, :],
                    )
                return w3

            w3s = {0: load_wo_slab(0), 1: load_wo_slab(1)}

            tail = []

            def emit_tail():
                yt, den, h = tail.pop(0)
                rinv = smallp.tile([P, QR], f32, tag="rq", name=f"rq{h}")
                nc.vector.reciprocal_approx_fast(rinv, den)
                nc.vector.tensor_tensor(yT[:, h, :], yt, rinv, op=OP.mult)

            def emit_ent(ent, yt, den, kv):
                """single: per-kt den + PV; pair: pair-summed den + 2 PV."""
                kind, m, n, pa, pb = ent
                if kind == "s":
                    kt = m
                    last = False
                    nc.tensor.matmul(
                        den[:, 0:n], ones128, pa,
                        start=(kt == 0), stop=False, skip_group_check=True,
                    )
                    nc.tensor.matmul(
                        yt[:, 0:n], vA[:, kv, kt, :], pa,
                        start=(kt == 0), stop=False, skip_group_check=True,
                    )
                else:
                    last = m == 7
                    pad = paddp.tile([P, QR], bf16, tag="pa", name=f"pad{m}")
                    nc.vector.tensor_tensor(pad[:, 0:n], pa, pb, op=OP.add)
                    nc.tensor.matmul(
                        den[:, 0:n], ones128, pad[:, 0:n],
                        start=False, stop=last, skip_group_check=True,
                    )
                    nc.tensor.matmul(
                        yt[:, 0:n], vA[:, kv, 2 * m, :], pa,
                        start=False, stop=False, skip_group_check=True,
                    )
                    nc.tensor.matmul(
                        yt[:, 0:n], vA[:, kv, 2 * m + 1, :], pb,
                        start=False, stop=last, skip_group_check=True,
                    )

            for h in range(NH):
                kv = h // (NH // NKV)
                yt = psY.tile([P, QR], f32, tag="Y", name=f"yt{h}")
                den = psD.tile([P, QR], f32, tag="D", name=f"den{h}")
                dq = []
                # kt 0..7: one kt per psum tile, causal mask accumulated
                # on the PE, pipeline depth 4
                for kt in range(8):
                    n = 64 * (8 - kt // 2)
                    i = (15 - kt) // 2
                    S = psS.tile([P, QR], f32, tag="S", name=f"S{h}_{kt}")
                    nc.tensor.matmul(
                        S[:, 0:n],
                        kT[:, kv, kt * P:(kt + 1) * P],
                        qT[:, h, 0:n],
                        start=True, stop=False, skip_group_check=True,
                    )
                    nc.tensor.matmul(
                        S[:, n - 64:n], ident, mk[:, i, kt % 2, :],
                        start=False, stop=True, skip_group_check=True,
                    )
                    if kt == 0 and tail:
                        emit_tail()
                    pt = ptpool.tile([P, QR], bf16, tag="pt", name=f"pt{h}_{kt}")
                    nc.scalar.activation(pt[:, 0:n], S[:, 0:n], AF.Exp)
                    dq.append(("s", kt, n, pt[:, 0:n], None))
                    if len(dq) > 3:
                        emit_ent(dq.pop(0), yt, den, kv)
                # kt 8..15: both kt of a pair packed into one psum bank
                for m in range(4, 8):
                    n = 64 * (8 - m)
                    i = 7 - m
                    S = psS.tile([P, QR], f32, tag="S", name=f"S{h}_{m}p")
                    for t in range(2):
                        nc.tensor.matmul(
                            S[:, t * n:(t + 1) * n],
                            kT[:, kv, (2 * m + t) * P:(2 * m + t + 1) * P],
                            qT[:, h, 0:n],
                            start=True, stop=False, skip_group_check=True,
                        )
                        nc.tensor.matmul(
                            S[:, (t + 1) * n - 64:(t + 1) * n], ident,
                            mk[:, i, t, :],
                            start=False, stop=True, skip_group_check=True,
                        )
                    pt = ptpool.tile([P, QR], bf16, tag="pt", name=f"pt{h}_{m}p")
                    nc.scalar.activation(pt[:, 0:2 * n], S[:, 0:2 * n], AF.Exp)
                    dq.append(("p", m, n, pt[:, 0:n], pt[:, n:2 * n]))
                    if len(dq) > 3:
                        emit_ent(dq.pop(0), yt, den, kv)
                while dq:
                    emit_ent(dq.pop(0), yt, den, kv)
                tail.append((yt, den, h))
            emit_tail()

            # ---------------- phase 4: output projection ----------------
            for s in range(4):
                w3 = w3s.pop(s)
                if s + 2 < 4:
                    w3s[s + 2] = load_wo_slab(s + 2)
                for rt in range(4):
                    ps = psS.tile([P, QR], f32, tag="S", name=f"o{s}_{rt}")
                    for h in range(NH):
                        nc.tensor.matmul(
                            ps, yT[:, h, rt * P:(rt + 1) * P], w3[:, h, :],
                            start=(h == 0), stop=(h == NH - 1),
                        )
                    ot = outpool.tile([P, QR], f32, tag="ot", name=f"ot{s}_{rt}")
                    nc.vector.tensor_copy(ot, ps)
                    nc.gpsimd.dma_start(
                        out=yo[rt * P:(rt + 1) * P, s * QR:(s + 1) * QR], in_=ot
                    )

    nc.compile()
    return nc


def _get_nc():
    if "nc" not in _CACHE:
        _CACHE["nc"] = _build()
    return _CACHE["nc"]


def _prep_in_maps(x, cos, sin, wq, wk, wv, wo):
    x = np.asarray(x, np.float32)
    cosr = np.asarray(cos, np.float32).reshape(T, HD // 2)
    sinr = np.asarray(sin, np.float32).reshape(T, HD // 2)
    # weight layouts match the SBUF tiles exactly -> contiguous DMAs
    wqb = np.ascontiguousarray(
        np.asarray(wq, np.float32).reshape(NCT, P, NH, HD)
        .transpose(2, 1, 0, 3).astype(BF16))
    wob = np.ascontiguousarray(
        np.asarray(wo, np.float32).reshape(NCT, P, 4, QR)
        .transpose(2, 1, 0, 3).astype(BF16))
    wkf = np.asarray(wk, np.float32)
    wvf = np.asarray(wv, np.float32)

    csf = np.ascontiguousarray(np.concatenate([cosr.T, sinr.T], axis=0).astype(BF16))
    snf = np.ascontiguousarray(np.concatenate([sinr.T, cosr.T], axis=0).astype(BF16))
    rpA_np, rpB_np = _rope_mats()

    maps = []
    for core in range(8):
        b, g = core // 4, core % 4
        qr = _qrows(g)
        xT_b = x[b].T
        maps.append({
            "xoT": np.ascontiguousarray(
                xT_b[:, qr].reshape(NCT, P, QR).transpose(1, 0, 2).astype(BF16)),
            "xfT": np.ascontiguousarray(
                xT_b.reshape(NCT, P, 4, QR).transpose(2, 1, 0, 3).astype(BF16)),
            "cso": np.ascontiguousarray(csf[:, qr]),
            "sno": np.ascontiguousarray(snf[:, qr]),
            "csf": csf,
            "snf": snf,
            "wq": wqb,
            "wks": np.ascontiguousarray(
                wkf[:, g * HD:(g + 1) * HD].reshape(NCT, P, HD)
                .transpose(1, 0, 2).astype(BF16)),
            "wvs": np.ascontiguousarray(
                wvf[:, g * HD:(g + 1) * HD].reshape(NCT, P, HD)
                .transpose(1, 0, 2).astype(BF16)),
            "wo": wob,
            "msk": np.ascontiguousarray(_mask01(g).transpose(2, 0, 1, 3)),
            "rpA": rpA_np,
            "rpB": rpB_np,
        })
    return maps


def kernel(x, cos, sin, wq, wk, wv, wo):
    from concourse.bass_utils import run_bass_kernel_spmd

    nc = _get_nc()
    maps = _prep_in_maps(x, cos, sin, wq, wk, wv, wo)
    _CACHE["in_maps"] = maps
    res = run_bass_kernel_spmd(nc, maps, list(range(8)))
    y = np.empty((B, T, C), np.float32)
    for core in range(8):
        b, g = core // 4, core % 4
        y[b][_qrows(g)] = res.results[core]["yo"]
    return y
